# revision 9
# baseline (speedup 1.0000x reference)
"""MLA-style causal self-attention on 8 Trainium2 NeuronCores.  v4

Sharding: tensor-parallel over heads (2 heads/core) for phases B/C/D; phase A
(latents = x @ W_qkv) is sharded over T: each core computes latents^T for its
own 256-wide T-chunk, ropes its local k_r slice, and two on-device AllGathers
(DRAM->Shared DRAM, bf16) replicate the latents: the c_q gather goes first so
phase B can start while the (smaller) kv gather completes under it. All matmul
operands are bf16 (same PE rate as f32r on TRN2, half the DMA/SBUF),
accumulation stays f32 in PSUM. Each core returns a partial out^T (mc-major
[16, 128, 2048] bf16); the host sums in f32.

v4 over v3:
 - C is software-pipelined: the den/attention-value matmuls for unit u run
   after the score matmuls of unit u+1, so the exp (ACT) and edge-mask (DVE)
   latency hides under the PE instead of stalling it.
 - The diagonal 512x512 block is computed as 6 half-units (256 queries) so
   only the valid causal triangle tiles run (saves 1 full unit per (h, i4)).
 - Per-phase PSUM pools; phase D double-buffers its 4 accumulators across
   8 banks so drain copies of chunk mc overlap the matmuls of mc+1.
 - cq_sb/kvg_sb are split into per-chunk tiles so phase B/C start as soon as
   their own readback DMA lands (tile-granular dependency tracking).
"""

import math
from contextlib import ExitStack

import numpy as np

import concourse.bass as bass
import concourse.tile as tile
from concourse import bacc, mybir
from concourse.bass_utils import run_bass_kernel_spmd
from concourse.masks import make_identity

F32 = mybir.dt.float32
BF16 = mybir.dt.bfloat16
AF = mybir.ActivationFunctionType

T_FULL = 2048
E = 2048
KV = 512
QL = 1024
RH = 64
QKH = KV + RH     # 576
NH = 16
NCORES = 8
HPC = NH // NCORES
SCALE = 1.0 / math.sqrt(float(KV))

P = 128
TC = T_FULL // NCORES  # 256
QLC = QL // P          # 8
KVC = KV // P          # 4
NKV = KVC + 1          # kv slots in the gather (4 c_kv + 1 padded rope)


def _make_rot64(nc, pool):
    rt0 = pool.tile([RH, RH], F32, tag="rt0")
    nc.gpsimd.memset(rt0[:], 0.0)
    nc.gpsimd.affine_select(
        out=rt0[:], in_=rt0[:], compare_op=mybir.AluOpType.not_equal,
        fill=1.0, base=-32, channel_multiplier=1, pattern=[[-1, RH]],
    )
    nc.gpsimd.affine_select(
        out=rt0[:], in_=rt0[:], compare_op=mybir.AluOpType.not_equal,
        fill=1.0, base=32, channel_multiplier=1, pattern=[[-1, RH]],
    )
    rt = pool.tile([RH, RH], BF16, tag="rt")
    nc.vector.tensor_copy(rt[:], rt0[:])
    return rt


def build_kernel(T=T_FULL):
    assert T % 512 == 0
    NT512 = T // 512
    NKT = T // P
    EK = E // P
    EH = EK // 2

    nc = bacc.Bacc("TRN2", target_bir_lowering=False, debug=False,
                   num_devices=NCORES)

    xT = nc.dram_tensor("xT", [P, EK, TC], BF16, kind="ExternalInput").ap()
    wcq = nc.dram_tensor("wcq", [P, EK, QL], BF16, kind="ExternalInput").ap()
    wkv = nc.dram_tensor("wkv", [P, EK, QKH], BF16, kind="ExternalInput").ap()
    wqd = nc.dram_tensor("wqd", [P, QLC, HPC * QKH], BF16,
                         kind="ExternalInput").ap()
    wo = nc.dram_tensor("wo", [E // P, P, HPC * KV // P, P], BF16,
                        kind="ExternalInput").ap()
    cosd = nc.dram_tensor("cosT", [RH, T], BF16, kind="ExternalInput").ap()
    sind = nc.dram_tensor("sinT", [RH, T], BF16, kind="ExternalInput").ap()
    cosl = nc.dram_tensor("cosl", [RH, TC], F32, kind="ExternalInput").ap()
    sinl = nc.dram_tensor("sinl", [RH, TC], F32, kind="ExternalInput").ap()
    outT = nc.dram_tensor("outT", [E // P, P, T], BF16,
                          kind="ExternalOutput").ap()

    with tile.TileContext(nc) as tc, ExitStack() as ctx:
        dram = ctx.enter_context(tc.tile_pool(name="dram", bufs=1, space="DRAM"))
        cst = ctx.enter_context(tc.tile_pool(name="cst", bufs=1))
        kvp = ctx.enter_context(tc.tile_pool(name="kvp", bufs=1))

        latloc_cq = dram.tile([P, QLC, TC], BF16, tag="latcq")
        latloc_kv = dram.tile([P, NKV, TC], BF16, tag="latkv")
        latg_cq = nc.dram_tensor("latg_cq", [NCORES, P, QLC, TC], BF16,
                                 kind="Internal", addr_space="Shared").ap()
        latg_kv = nc.dram_tensor("latg_kv", [NCORES, P, NKV, TC], BF16,
                                 kind="Internal", addr_space="Shared").ap()

        # ---- global constants ----
        ident0 = cst.tile([P, P], F32, tag="ident0")
        make_identity(nc, ident0[:])
        ident = cst.tile([P, P], BF16, tag="ident")
        nc.vector.tensor_copy(ident[:], ident0[:])
        ones0 = cst.tile([P, 1], F32, tag="ones0")
        nc.gpsimd.memset(ones0[:], 1.0)
        ones_col = cst.tile([P, 1], BF16, tag="ones")
        nc.vector.tensor_copy(ones_col[:], ones0[:])
        rt = _make_rot64(nc, cst)
        # diagonal half-masks [128, 256]: dmask0: keep k <= q; dmask1: keep
        # k + 128 <= q  (q = local query col, k = key channel)
        dmasks = []
        with tc.tile_pool(name="mtmp", bufs=1) as mtmp:
            for r in range(2):
                m0 = mtmp.tile([P, 256], F32, tag=f"m{r}", name=f"m{r}")
                nc.gpsimd.memset(m0[:], 1.0)
                nc.gpsimd.affine_select(
                    out=m0[:], in_=m0[:], compare_op=mybir.AluOpType.is_ge,
                    fill=0.0, base=-P * r, channel_multiplier=-1,
                    pattern=[[1, 256]],
                )
                mb = cst.tile([P, 256], BF16, tag=f"dmask{r}",
                              name=f"dmask{r}")
                nc.vector.tensor_copy(mb[:], m0[:])
                dmasks.append(mb)

        # ================= Phase A: own latents^T chunk ====================
        # kv groups run FIRST so the kv gather hides under the c_q compute;
        # the (bigger) c_q gather is split in two so its first half starts at
        # the c_q midpoint. Loads are kc-split across queues so the first
        # matmul starts after ~0.5MB instead of the full weight load.
        latloc_cqa = dram.tile([P, QLC // 2, TC], BF16, tag="latcqa")
        latloc_cqb = dram.tile([P, QLC // 2, TC], BF16, tag="latcqb")
        latg_cqa = nc.dram_tensor("latg_cqa", [NCORES, P, QLC // 2, TC], BF16,
                                  kind="Internal", addr_space="Shared").ap()
        latg_cqb = nc.dram_tensor("latg_cqb", [NCORES, P, QLC // 2, TC], BF16,
                                  kind="Internal", addr_space="Shared").ap()
        with ExitStack() as actx:
            aw = actx.enter_context(tc.tile_pool(name="aw", bufs=1))
            asr = actx.enter_context(tc.tile_pool(name="asr", bufs=1))
            astp = actx.enter_context(tc.tile_pool(name="astp", bufs=2))
            ppA = actx.enter_context(tc.tile_pool(name="ppA", bufs=2,
                                                  space="PSUM"))

            EQ = EK // 4  # 4-way kc split of the loads
            cosls = asr.tile([RH, TC], F32, tag="cosls")
            nc.scalar.dma_start(cosls[:], cosl[:])
            sinls = asr.tile([RH, TC], F32, tag="sinls")
            nc.scalar.dma_start(sinls[:], sinl[:])
            xt4 = []
            wkv4 = []
            wcq4 = []
            for qi in range(4):
                ksl = slice(qi * EQ, (qi + 1) * EQ)
                xt = asr.tile([P, EQ, TC], BF16, tag=f"xt4_{qi}",
                              name=f"xt4_{qi}")
                nc.sync.dma_start(xt[:], xT[:, ksl, :])
                xt4.append(xt)
                wk = aw.tile([P, EQ, QKH], BF16, tag=f"wkv4_{qi}",
                             name=f"wkv4_{qi}")
                nc.scalar.dma_start(wk[:], wkv[:, ksl, :])
                wkv4.append(wk)
            for qi in range(4):
                ksl = slice(qi * EQ, (qi + 1) * EQ)
                wc = aw.tile([P, EQ, QL], BF16, tag=f"wcq4_{qi}",
                             name=f"wcq4_{qi}")
                eng = [nc.gpsimd, nc.scalar, nc.sync, nc.sync][qi]
                eng.dma_start(wc[:], wcq[:, ksl, :])
                wcq4.append(wc)

            def amm(ps, weights, c0, M, ps_off=0):
                for kc in range(EK):
                    qi, kk = kc // EQ, kc % EQ
                    nc.tensor.matmul(
                        ps[ps_off : ps_off + M],
                        weights[qi][:, kk, c0 : c0 + M], xt4[qi][:, kk, :],
                        start=(kc == 0), stop=(kc == EK - 1))

            # kv + rope first
            st_kv = asr.tile([P, NKV, TC], BF16, tag="st_kv")
            for dc in range(KVC):
                ps = ppA.tile([P, TC], F32, tag="mm", name="psA")
                amm(ps, wkv4, dc * P, P)
                nc.vector.tensor_copy(st_kv[:, dc, :], ps[:])
            ps = ppA.tile([P, TC], F32, tag="mm", name="psA")
            amm(ps, wkv4, KV, RH)
            kr = astp.tile([RH, TC], BF16, tag="kr", name="kr")
            nc.vector.tensor_copy(kr[:], ps[:RH])
            pr = ppA.tile([RH, TC], F32, tag="mm", name="prk")
            nc.tensor.matmul(pr[:], rt[:], kr[:], start=True, stop=True)
            kr2 = astp.tile([RH, TC], F32, tag="kr2", name="kr2")
            nc.vector.tensor_mul(kr2[:], ps[:RH], cosls[:])
            rot = astp.tile([RH, TC], F32, tag="rot", name="rotk")
            nc.vector.tensor_mul(rot[:], pr[:], sinls[:])
            nc.vector.tensor_add(st_kv[:RH, KVC, :], kr2[:], rot[:])
            nc.scalar.dma_start(latloc_kv[:], st_kv[:])
            nc.gpsimd.collective_compute(
                "AllGather", mybir.AluOpType.bypass,
                replica_groups=[list(range(NCORES))],
                ins=[latloc_kv[:]], outs=[latg_kv[:]],
            )

            # c_q groups, first half -> gather, second half -> gather
            st_cqa = asr.tile([P, QLC // 2, TC], BF16, tag="st_cqa")
            for gi in range(QLC // 2):
                ps = ppA.tile([P, TC], F32, tag="mm", name="psA")
                amm(ps, wcq4, gi * P, P)
                nc.vector.tensor_copy(st_cqa[:, gi, :], ps[:])
            nc.scalar.dma_start(latloc_cqa[:], st_cqa[:])
            nc.gpsimd.collective_compute(
                "AllGather", mybir.AluOpType.bypass,
                replica_groups=[list(range(NCORES))],
                ins=[latloc_cqa[:]], outs=[latg_cqa[:]],
            )
            st_cqb = asr.tile([P, QLC // 2, TC], BF16, tag="st_cqb")
            for gi in range(QLC // 2):
                ps = ppA.tile([P, TC], F32, tag="mm", name="psA")
                amm(ps, wcq4, KV + gi * P, P)
                nc.vector.tensor_copy(st_cqb[:, gi, :], ps[:])
            nc.scalar.dma_start(latloc_cqb[:], st_cqb[:])
            nc.gpsimd.collective_compute(
                "AllGather", mybir.AluOpType.bypass,
                replica_groups=[list(range(NCORES))],
                ins=[latloc_cqb[:]], outs=[latg_cqb[:]],
            )

        # ---- residents for B/C/D (allocated after phase A frees its pools;
        # the DMAs load during the gather window) ----
        cq4a = [kvp.tile([P, 2, QLC // 2, TC], BF16, tag=f"cq4a_{i}",
                         name=f"cq4a_{i}") for i in range(4)]
        cq4b = [kvp.tile([P, 2, QLC // 2, TC], BF16, tag=f"cq4b_{i}",
                         name=f"cq4b_{i}") for i in range(4)]
        kvg4 = [kvp.tile([P, 2, NKV, TC], BF16, tag=f"kvg4_{i}",
                         name=f"kvg4_{i}") for i in range(4)]
        v = kvp.tile([P, NKT, KV], BF16, tag="v")
        yT = kvp.tile([P, HPC * KV // P, T], BF16, tag="yT")
        cosT = kvp.tile([RH, T], BF16, tag="cosT")
        nc.sync.dma_start(cosT[:], cosd[:])
        ssinT = kvp.tile([RH, T], BF16, tag="ssinT")
        nc.sync.dma_start(ssinT[:], sind[:])
        wqds = kvp.tile([P, QLC, HPC * QKH], BF16, tag="wqds")
        nc.scalar.dma_start(wqds[:], wqd[:])

        # ================= readback (big-line DMAs) =======================
        for i in range(4):
            nc.scalar.dma_start(
                kvg4[i][:],
                latg_kv[2 * i : 2 * i + 2].rearrange("r p g t -> p r g t"),
            )
        for i in range(4):
            nc.sync.dma_start(
                cq4a[i][:],
                latg_cqa[2 * i : 2 * i + 2].rearrange("r p ko t -> p r ko t"),
            )
            nc.scalar.dma_start(
                cq4b[i][:],
                latg_cqb[2 * i : 2 * i + 2].rearrange("r p ko t -> p r ko t"),
            )

        def kslice(j):
            """(tile, rank-in-tile, key-col slice) for key tile j."""
            jr, jh = j // 2, j % 2
            return kvg4[jr // 2], jr % 2, slice(jh * P, (jh + 1) * P)

        # ============ Phases preC + fused B/C =============================
        with ExitStack() as bctx:
            bcs = bctx.enter_context(tc.tile_pool(name="bcs", bufs=2))
            pp = bctx.enter_context(tc.tile_pool(name="pp", bufs=2,
                                                 space="PSUM"))
            ppy = bctx.enter_context(tc.tile_pool(name="ppy", bufs=1,
                                                  space="PSUM"))
            pden = bctx.enter_context(tc.tile_pool(name="pden", bufs=1,
                                                   space="PSUM"))
            ptr = bctx.enter_context(tc.tile_pool(name="ptr", bufs=1,
                                                  space="PSUM"))

            def transpose_batch(tts):
                """v[t, d] via PE transposes of c_kv^T for key tiles tts.
                Emitted just before the first C chunk that needs them so the
                in-order PE stream doesn't stall on the kv gather before B."""
                for tt in tts:
                    kt, kr_i, ksl128 = kslice(tt)
                    for dc in range(KVC):
                        pt = ptr.tile([P, P], BF16, tag="tr", name="pt")
                        nc.tensor.transpose(
                            pt[:], kt[:, kr_i, dc, ksl128], ident[:]
                        )
                        nc.vector.tensor_copy(
                            v[:, tt, dc * P : (dc + 1) * P], pt[:]
                        )

            qgroups = [(KV, RH)] + [(i * P, P) for i in range(KVC)]
            for h in range(HPC):
                for i4 in range(NT512):
                    qsl = slice(i4 * 512, (i4 + 1) * 512)
                    # ---- B chunk: q^T for queries i4 (SCALE folded in) ----
                    qTc = [bcs.tile([P, 512], BF16, tag=f"qTc{i}",
                                    name=f"qTc{i}") for i in range(KVC)]
                    qrRaw = bcs.tile([RH, 512], BF16, tag="qrRaw", name="qrRaw")
                    qrT = bcs.tile([RH, 512], BF16, tag="qrT", name="qrT")
                    for (m0, M) in qgroups:
                        ps = pp.tile([P, 512], F32, tag="mm", name="psB")
                        for kc in range(QLC):
                            cqs = (cq4a[i4][:, :, kc, :] if kc < QLC // 2
                                   else cq4b[i4][:, :, kc - QLC // 2, :])
                            nc.tensor.matmul(
                                ps[:M], wqds[:, kc, h * QKH + m0 :
                                             h * QKH + m0 + M],
                                cqs,
                                start=(kc == 0), stop=(kc == QLC - 1),
                            )
                        if m0 < KV:
                            nc.vector.tensor_scalar_mul(
                                qTc[m0 // P][:], ps[:], SCALE
                            )
                        else:
                            nc.vector.tensor_scalar_mul(qrRaw[:], ps[:RH], SCALE)
                            pr = pp.tile([RH, 512], F32, tag="mm", name="prq")
                            nc.tensor.matmul(pr[:], rt[:], qrRaw[:],
                                             start=True, stop=True)
                            nc.vector.tensor_mul(qrT[:], qrRaw[:], cosT[:, qsl])
                            rot = bcs.tile([RH, 512], BF16, tag="rot",
                                           name="rotq")
                            nc.vector.tensor_mul(rot[:], pr[:], ssinT[:, qsl])
                            nc.vector.tensor_add(qrT[:], qrT[:], rot[:])

                    # ---- C chunk: causal attention, software-pipelined ----
                    if h == 0:
                        transpose_batch(range(4 * i4, 4 * i4 + 4))
                    # units: (key tile j, qlo, qw, mask, first, last)
                    nj_off = 4 * i4
                    units = []
                    for j in range(nj_off):
                        units.append((j, 0, 512, None, j == 0, False))
                    units.append((nj_off + 0, 0, 256, dmasks[0],
                                  nj_off == 0, False))
                    units.append((nj_off + 1, 0, 256, dmasks[1], False, True))
                    units.append((nj_off + 0, 256, 256, None,
                                  nj_off == 0, False))
                    units.append((nj_off + 1, 256, 256, None, False, False))
                    units.append((nj_off + 2, 256, 256, dmasks[0],
                                  False, False))
                    units.append((nj_off + 3, 256, 256, dmasks[1],
                                  False, True))
                    psden = pden.tile([1, 512], F32, tag="den", name="psden")
                    psy = [ppy.tile([P, 512], F32, tag=f"y{dc}",
                                    name=f"psy{dc}")
                           for dc in range(KVC)]

                    def dav(pu):
                        (j, qlo, qw, _m, first, last), se = pu
                        qs = slice(qlo, qlo + qw)
                        nc.tensor.matmul(
                            psden[:, qs], ones_col[:], se[:, 0:qw],
                            start=first, stop=last, skip_group_check=True,
                        )
                        for dc in range(KVC):
                            nc.tensor.matmul(
                                psy[dc][:, qs],
                                v[:, j, dc * P : (dc + 1) * P], se[:, 0:qw],
                                start=first, stop=last, skip_group_check=True,
                            )

                    pend = None
                    for u in units:
                        (j, qlo, qw, mask, first, last) = u
                        kt, kr_i, ksl = kslice(j)
                        qs = slice(qlo, qlo + qw)
                        ps = pp.tile([P, 512], F32, tag="mm", name="psS")
                        for dc in range(KVC):
                            nc.tensor.matmul(
                                ps[:, 0:qw], kt[:, kr_i, dc, ksl],
                                qTc[dc][:, qs],
                                start=(dc == 0), stop=False,
                            )
                        nc.tensor.matmul(
                            ps[:, 0:qw], kt[:RH, kr_i, KVC, ksl], qrT[:, qs],
                            start=False, stop=True,
                        )
                        se = bcs.tile([P, 512], BF16, tag="se", bufs=3,
                                      name="se")
                        nc.scalar.activation(se[:, 0:qw], ps[:, 0:qw], AF.Exp)
                        if mask is not None:
                            nc.vector.tensor_mul(se[:, 0:qw], se[:, 0:qw],
                                                 mask[:])
                        if pend is not None:
                            dav(pend)
                        pend = (u, se)
                    dav(pend)

                    deninv = bcs.tile([1, 512], F32, tag="deninv",
                                      name="deninv")
                    nc.vector.reciprocal_approx_fast(out=deninv[:],
                                                     in_=psden[:])
                    denb = bcs.tile([P, 512], F32, tag="denb", name="denb")
                    nc.gpsimd.partition_broadcast(denb[:], deninv[:])
                    for dc in range(KVC):
                        nc.vector.tensor_mul(
                            yT[:, h * KVC + dc, qsl], psy[dc][:], denb[:]
                        )

        # ================= Phase D: out^T = W_out_c^T @ y^T ===============
        with ExitStack() as dctx:
            dwp = dctx.enter_context(tc.tile_pool(name="dwp", bufs=2))
            dst = dctx.enter_context(tc.tile_pool(name="dst", bufs=2))
            ppD = dctx.enter_context(tc.tile_pool(name="ppD", bufs=2,
                                                  space="PSUM"))

            DK = HPC * KV // P  # 8 contraction chunks
            for mc in range(E // P):
                wot = dwp.tile([P, DK, P], BF16, tag="wo", name="wo")
                nc.sync.dma_start(wot[:], wo[mc])
                psD = [ppD.tile([P, 512], F32, tag=f"d{tcc}", name=f"psD{tcc}")
                       for tcc in range(NT512)]
                for kc in range(DK):
                    for tcc in range(NT512):
                        nc.tensor.matmul(
                            psD[tcc][:], wot[:, kc, :],
                            yT[:, kc, tcc * 512 : (tcc + 1) * 512],
                            start=(kc == 0), stop=(kc == DK - 1),
                        )
                ost = dst.tile([P, T], BF16, tag="ost", name="ost")
                for tcc in range(NT512):
                    osl = slice(tcc * 512, (tcc + 1) * 512)
                    if tcc % 2 == 0:
                        nc.vector.tensor_copy(ost[:, osl], psD[tcc][:])
                    else:
                        nc.scalar.copy(ost[:, osl], psD[tcc][:])
                eng = nc.gpsimd if mc % 2 == 0 else nc.scalar
                eng.dma_start(outT[mc], ost[:])

    nc.compile()
    return nc


_NC_CACHE = {}


def _get_nc(T=T_FULL):
    if T not in _NC_CACHE:
        _NC_CACHE[T] = build_kernel(T)
    return _NC_CACHE[T]


def _swizzle_k(w, p=P):
    K, M = w.shape
    return np.ascontiguousarray(w.reshape(K // p, p, M).transpose(1, 0, 2))


def make_in_maps(x, cos, sin, W_qkv, W_qdec, W_out):
    import ml_dtypes

    bf16 = ml_dtypes.bfloat16
    xT = np.ascontiguousarray(np.asarray(x)[0].T).astype(bf16)
    cosT = np.ascontiguousarray(np.asarray(cos).T.astype(np.float32))
    sinT = np.ascontiguousarray(np.asarray(sin).T.astype(np.float32))
    sinN = sinT.copy()
    sinN[: RH // 2, :] *= -1.0
    W_qkv = np.asarray(W_qkv).astype(bf16)
    wcq = _swizzle_k(W_qkv[:, QKH:])
    wkv = _swizzle_k(W_qkv[:, :QKH])
    W_qdec = np.asarray(W_qdec)
    W_out = np.asarray(W_out)
    in_maps = []
    for c in range(NCORES):
        tsl = slice(c * TC, (c + 1) * TC)
        wos = W_out[c * HPC * KV : (c + 1) * HPC * KV].astype(bf16)
        wos = _swizzle_k(wos)                      # [128, 8, 2048]
        wos = wos.reshape(P, HPC * KV // P, E // P, P)
        wos = np.ascontiguousarray(wos.transpose(2, 0, 1, 3))  # [16,128,8,128]
        in_maps.append({
            "xT": _swizzle_k(np.ascontiguousarray(xT[:, tsl])),
            "wcq": wcq,
            "wkv": wkv,
            "wqd": _swizzle_k(np.ascontiguousarray(
                W_qdec[:, c * HPC * QKH : (c + 1) * HPC * QKH]).astype(bf16)),
            "wo": wos,
            "cosT": cosT.astype(bf16),
            "sinT": sinN.astype(bf16),
            "cosl": np.ascontiguousarray(cosT[:, tsl]),
            "sinl": np.ascontiguousarray(sinN[:, tsl]),
        })
    return in_maps


def kernel(x, cos, sin, W_qkv, W_qdec, W_out, _trace=False, _tmpdir=None):
    T = np.asarray(x).shape[1]
    nc = _get_nc(T)
    in_maps = make_in_maps(x, cos, sin, W_qkv, W_qdec, W_out)
    res = run_bass_kernel_spmd(
        nc, in_maps, core_ids=list(range(NCORES)),
        trace=_trace, tmpdir=_tmpdir,
    )
    out = np.zeros((E, T), np.float32)
    for r in res.results:
        out += np.asarray(r["outT"], dtype=np.float32).reshape(E, T)
    kernel.last_results = res
    return np.ascontiguousarray(out.T)[None].astype(np.float32)


# revision 14
# speedup vs baseline: 1.0625x; 1.0625x over previous
"""MLA-style causal self-attention on 8 Trainium2 NeuronCores.  v4

Sharding: tensor-parallel over heads (2 heads/core) for phases B/C/D; phase A
(latents = x @ W_qkv) is sharded over T: each core computes latents^T for its
own 256-wide T-chunk, ropes its local k_r slice, and two on-device AllGathers
(DRAM->Shared DRAM, bf16) replicate the latents: the c_q gather goes first so
phase B can start while the (smaller) kv gather completes under it. All matmul
operands are bf16 (same PE rate as f32r on TRN2, half the DMA/SBUF),
accumulation stays f32 in PSUM. Each core returns a partial out^T (mc-major
[16, 128, 2048] bf16); the host sums in f32.

v4 over v3:
 - C is software-pipelined: the den/attention-value matmuls for unit u run
   after the score matmuls of unit u+1, so the exp (ACT) and edge-mask (DVE)
   latency hides under the PE instead of stalling it.
 - The diagonal 512x512 block is computed as 6 half-units (256 queries) so
   only the valid causal triangle tiles run (saves 1 full unit per (h, i4)).
 - Per-phase PSUM pools; phase D double-buffers its 4 accumulators across
   8 banks so drain copies of chunk mc overlap the matmuls of mc+1.
 - cq_sb/kvg_sb are split into per-chunk tiles so phase B/C start as soon as
   their own readback DMA lands (tile-granular dependency tracking).
"""

import math
from contextlib import ExitStack

import numpy as np

import concourse.bass as bass
import concourse.tile as tile
from concourse import bacc, mybir
from concourse.bass_utils import run_bass_kernel_spmd
from concourse.masks import make_identity

F32 = mybir.dt.float32
BF16 = mybir.dt.bfloat16
AF = mybir.ActivationFunctionType

T_FULL = 2048
E = 2048
KV = 512
QL = 1024
RH = 64
QKH = KV + RH     # 576
NH = 16
NCORES = 8
HPC = NH // NCORES
SCALE = 1.0 / math.sqrt(float(KV))

P = 128
TC = T_FULL // NCORES  # 256
QLC = QL // P          # 8
KVC = KV // P          # 4
NKV = KVC + 1          # kv slots in the gather (4 c_kv + 1 padded rope)


def _make_rot64(nc, pool):
    rt0 = pool.tile([RH, RH], F32, tag="rt0")
    nc.gpsimd.memset(rt0[:], 0.0)
    nc.gpsimd.affine_select(
        out=rt0[:], in_=rt0[:], compare_op=mybir.AluOpType.not_equal,
        fill=1.0, base=-32, channel_multiplier=1, pattern=[[-1, RH]],
    )
    nc.gpsimd.affine_select(
        out=rt0[:], in_=rt0[:], compare_op=mybir.AluOpType.not_equal,
        fill=1.0, base=32, channel_multiplier=1, pattern=[[-1, RH]],
    )
    rt = pool.tile([RH, RH], BF16, tag="rt")
    nc.vector.tensor_copy(rt[:], rt0[:])
    return rt


def build_kernel(T=T_FULL):
    assert T % 512 == 0
    NT512 = T // 512
    NKT = T // P
    EK = E // P
    EH = EK // 2

    nc = bacc.Bacc("TRN2", target_bir_lowering=False, debug=False,
                   num_devices=NCORES)

    xT = nc.dram_tensor("xT", [P, EK, TC], BF16, kind="ExternalInput").ap()
    wcq = nc.dram_tensor("wcq", [P, EK, QL], BF16, kind="ExternalInput").ap()
    wkv = nc.dram_tensor("wkv", [P, EK, QKH], BF16, kind="ExternalInput").ap()
    wqd = nc.dram_tensor("wqd", [P, QLC, HPC * QKH], BF16,
                         kind="ExternalInput").ap()
    wo = nc.dram_tensor("wo", [E // P, P, HPC * KV // P, P], BF16,
                        kind="ExternalInput").ap()
    cosd = nc.dram_tensor("cosT", [RH, T], BF16, kind="ExternalInput").ap()
    sind = nc.dram_tensor("sinT", [RH, T], BF16, kind="ExternalInput").ap()
    cosl = nc.dram_tensor("cosl", [RH, TC], F32, kind="ExternalInput").ap()
    sinl = nc.dram_tensor("sinl", [RH, TC], F32, kind="ExternalInput").ap()
    outT = nc.dram_tensor("outT", [E // P, P, T], BF16,
                          kind="ExternalOutput").ap()

    with tile.TileContext(nc) as tc, ExitStack() as ctx:
        dram = ctx.enter_context(tc.tile_pool(name="dram", bufs=1, space="DRAM"))
        cst = ctx.enter_context(tc.tile_pool(name="cst", bufs=1))
        kvp = ctx.enter_context(tc.tile_pool(name="kvp", bufs=1))

        latloc_cq = dram.tile([P, QLC, TC], BF16, tag="latcq")
        latloc_kv = dram.tile([P, NKV, TC], BF16, tag="latkv")
        latg_cq = nc.dram_tensor("latg_cq", [NCORES, P, QLC, TC], BF16,
                                 kind="Internal", addr_space="Shared").ap()
        latg_kv = nc.dram_tensor("latg_kv", [NCORES, P, NKV, TC], BF16,
                                 kind="Internal", addr_space="Shared").ap()

        # ---- global constants ----
        ident0 = cst.tile([P, P], F32, tag="ident0")
        make_identity(nc, ident0[:])
        ident = cst.tile([P, P], BF16, tag="ident")
        nc.vector.tensor_copy(ident[:], ident0[:])
        ones0 = cst.tile([P, 1], F32, tag="ones0")
        nc.gpsimd.memset(ones0[:], 1.0)
        ones_col = cst.tile([P, 1], BF16, tag="ones")
        nc.vector.tensor_copy(ones_col[:], ones0[:])
        rt = _make_rot64(nc, cst)
        # diagonal half-masks [128, 256]: dmask0: keep k <= q; dmask1: keep
        # k + 128 <= q  (q = local query col, k = key channel)
        dmasks = []
        with tc.tile_pool(name="mtmp", bufs=1) as mtmp:
            for r in range(2):
                m0 = mtmp.tile([P, 256], F32, tag=f"m{r}", name=f"m{r}")
                nc.gpsimd.memset(m0[:], 1.0)
                nc.gpsimd.affine_select(
                    out=m0[:], in_=m0[:], compare_op=mybir.AluOpType.is_ge,
                    fill=0.0, base=-P * r, channel_multiplier=-1,
                    pattern=[[1, 256]],
                )
                mb = cst.tile([P, 256], BF16, tag=f"dmask{r}",
                              name=f"dmask{r}")
                nc.vector.tensor_copy(mb[:], m0[:])
                dmasks.append(mb)

        # ================= Phase A: own latents^T chunk ====================
        # kv groups run FIRST so the kv gather (which has the ~20us
        # first-collective sync cost) hides under the c_q compute; the c_q
        # gather follows it on the ring. Loads are kc-split across queues so
        # the first matmul starts after ~0.5MB instead of the full load.
        with ExitStack() as actx:
            aw = actx.enter_context(tc.tile_pool(name="aw", bufs=1))
            asr = actx.enter_context(tc.tile_pool(name="asr", bufs=1))
            astp = actx.enter_context(tc.tile_pool(name="astp", bufs=2))
            ppA = actx.enter_context(tc.tile_pool(name="ppA", bufs=2,
                                                  space="PSUM"))

            EQ = EK // 4  # 4-way kc split of the loads
            cosls = asr.tile([RH, TC], F32, tag="cosls")
            nc.scalar.dma_start(cosls[:], cosl[:])
            sinls = asr.tile([RH, TC], F32, tag="sinls")
            nc.scalar.dma_start(sinls[:], sinl[:])
            xt4 = []
            wkv4 = []
            wcq4 = []
            for qi in range(4):
                ksl = slice(qi * EQ, (qi + 1) * EQ)
                xt = asr.tile([P, EQ, TC], BF16, tag=f"xt4_{qi}",
                              name=f"xt4_{qi}")
                nc.sync.dma_start(xt[:], xT[:, ksl, :])
                xt4.append(xt)
                wk = aw.tile([P, EQ, QKH], BF16, tag=f"wkv4_{qi}",
                             name=f"wkv4_{qi}")
                nc.scalar.dma_start(wk[:], wkv[:, ksl, :])
                wkv4.append(wk)
            for qi in range(4):
                ksl = slice(qi * EQ, (qi + 1) * EQ)
                wc = aw.tile([P, EQ, QL], BF16, tag=f"wcq4_{qi}",
                             name=f"wcq4_{qi}")
                eng = [nc.gpsimd, nc.scalar, nc.sync, nc.scalar][qi]
                eng.dma_start(wc[:], wcq[:, ksl, :])
                wcq4.append(wc)

            def amm(ps, weights, c0, M, ps_off=0):
                for kc in range(EK):
                    qi, kk = kc // EQ, kc % EQ
                    nc.tensor.matmul(
                        ps[ps_off : ps_off + M],
                        weights[qi][:, kk, c0 : c0 + M], xt4[qi][:, kk, :],
                        start=(kc == 0), stop=(kc == EK - 1))

            # kv + rope first
            st_kv = asr.tile([P, NKV, TC], BF16, tag="st_kv")
            for dc in range(KVC):
                ps = ppA.tile([P, TC], F32, tag="mm", name="psA")
                amm(ps, wkv4, dc * P, P)
                nc.vector.tensor_copy(st_kv[:, dc, :], ps[:])
            ps = ppA.tile([P, TC], F32, tag="mm", name="psA")
            amm(ps, wkv4, KV, RH)
            kr = astp.tile([RH, TC], BF16, tag="kr", name="kr")
            nc.vector.tensor_copy(kr[:], ps[:RH])
            pr = ppA.tile([RH, TC], F32, tag="mm", name="prk")
            nc.tensor.matmul(pr[:], rt[:], kr[:], start=True, stop=True)
            kr2 = astp.tile([RH, TC], F32, tag="kr2", name="kr2")
            nc.vector.tensor_mul(kr2[:], ps[:RH], cosls[:])
            rot = astp.tile([RH, TC], F32, tag="rot", name="rotk")
            nc.vector.tensor_mul(rot[:], pr[:], sinls[:])
            nc.vector.tensor_add(st_kv[:RH, KVC, :], kr2[:], rot[:])
            nc.scalar.dma_start(latloc_kv[:], st_kv[:])
            nc.gpsimd.collective_compute(
                "AllGather", mybir.AluOpType.bypass,
                replica_groups=[list(range(NCORES))],
                ins=[latloc_kv[:]], outs=[latg_kv[:]],
            )

            # c_q groups -> one gather (runs on the ring after the kv one)
            st_cq = asr.tile([P, QLC, TC], BF16, tag="st_cq")
            for gi in range(QLC):
                ps = ppA.tile([P, TC], F32, tag="mm", name="psA")
                amm(ps, wcq4, gi * P, P)
                nc.vector.tensor_copy(st_cq[:, gi, :], ps[:])
            nc.scalar.dma_start(latloc_cq[:], st_cq[:])
            nc.gpsimd.collective_compute(
                "AllGather", mybir.AluOpType.bypass,
                replica_groups=[list(range(NCORES))],
                ins=[latloc_cq[:]], outs=[latg_cq[:]],
            )

        # ---- residents for B/C/D (allocated after phase A frees its pools;
        # the DMAs load during the gather window) ----
        cq4 = [kvp.tile([P, 2, QLC, TC], BF16, tag=f"cq4_{i}",
                        name=f"cq4_{i}") for i in range(4)]
        kvg4 = [kvp.tile([P, 2, NKV, TC], BF16, tag=f"kvg4_{i}",
                         name=f"kvg4_{i}") for i in range(4)]
        v = kvp.tile([P, NKT, KV], BF16, tag="v")
        yT = kvp.tile([P, HPC * KV // P, T], BF16, tag="yT")
        cosT = kvp.tile([RH, T], BF16, tag="cosT")
        nc.sync.dma_start(cosT[:], cosd[:])
        ssinT = kvp.tile([RH, T], BF16, tag="ssinT")
        nc.sync.dma_start(ssinT[:], sind[:])
        wqds = kvp.tile([P, QLC, HPC * QKH], BF16, tag="wqds")
        nc.scalar.dma_start(wqds[:], wqd[:])

        # ================= readback (big-line DMAs) =======================
        # kvg on scalar right after wqds (all land well before their C units);
        # cq on sync, one tile per 512-query chunk
        for i in range(4):
            nc.scalar.dma_start(
                kvg4[i][:],
                latg_kv[2 * i : 2 * i + 2].rearrange("r p g t -> p r g t"),
            )
        for i in range(4):
            nc.sync.dma_start(
                cq4[i][:],
                latg_cq[2 * i : 2 * i + 2].rearrange("r p ko t -> p r ko t"),
            )

        def kslice(j):
            """(tile, rank-in-tile, key-col slice) for key tile j."""
            jr, jh = j // 2, j % 2
            return kvg4[jr // 2], jr % 2, slice(jh * P, (jh + 1) * P)

        # ============ Phases preC + fused B/C =============================
        with ExitStack() as bctx:
            bcs = bctx.enter_context(tc.tile_pool(name="bcs", bufs=2))
            pp = bctx.enter_context(tc.tile_pool(name="pp", bufs=2,
                                                 space="PSUM"))
            ppy = bctx.enter_context(tc.tile_pool(name="ppy", bufs=1,
                                                  space="PSUM"))
            pden = bctx.enter_context(tc.tile_pool(name="pden", bufs=1,
                                                   space="PSUM"))
            ptr = bctx.enter_context(tc.tile_pool(name="ptr", bufs=1,
                                                  space="PSUM"))

            def transpose_batch(tts):
                """v[t, d] via PE transposes of c_kv^T for key tiles tts.
                Emitted just before the first C chunk that needs them so the
                in-order PE stream doesn't stall on the kv gather before B."""
                for tt in tts:
                    kt, kr_i, ksl128 = kslice(tt)
                    for dc in range(KVC):
                        pt = ptr.tile([P, P], BF16, tag="tr", name="pt")
                        nc.tensor.transpose(
                            pt[:], kt[:, kr_i, dc, ksl128], ident[:]
                        )
                        nc.vector.tensor_copy(
                            v[:, tt, dc * P : (dc + 1) * P], pt[:]
                        )

            # rope group SECOND: its DVE chain (scale+rotate muls) queues
            # behind the previous drain's yT muls, so give the DVE one nope
            # group of headroom before the rt matmul needs qrRaw
            qgroups = [(0, P), (KV, RH)] + [(i * P, P) for i in range(1, KVC)]
            for h in range(HPC):
                for i4 in range(NT512):
                    qsl = slice(i4 * 512, (i4 + 1) * 512)
                    # ---- B chunk: q^T for queries i4 (SCALE folded in) ----
                    qTc = [bcs.tile([P, 512], BF16, tag=f"qTc{i}",
                                    name=f"qTc{i}") for i in range(KVC)]
                    qrRaw = bcs.tile([RH, 512], BF16, tag="qrRaw", name="qrRaw")
                    qrT = bcs.tile([RH, 512], BF16, tag="qrT", name="qrT")
                    for (m0, M) in qgroups:
                        ps = pp.tile([P, 512], F32, tag="mm", name="psB")
                        for kc in range(QLC):
                            nc.tensor.matmul(
                                ps[:M], wqds[:, kc, h * QKH + m0 :
                                             h * QKH + m0 + M],
                                cq4[i4][:, :, kc, :],
                                start=(kc == 0), stop=(kc == QLC - 1),
                            )
                        if m0 < KV:
                            nc.vector.tensor_scalar_mul(
                                qTc[m0 // P][:], ps[:], SCALE
                            )
                        else:
                            nc.vector.tensor_scalar_mul(qrRaw[:], ps[:RH], SCALE)
                            pr = pp.tile([RH, 512], F32, tag="mm", name="prq")
                            nc.tensor.matmul(pr[:], rt[:], qrRaw[:],
                                             start=True, stop=True)
                            nc.vector.tensor_mul(qrT[:], qrRaw[:], cosT[:, qsl])
                            rot = bcs.tile([RH, 512], BF16, tag="rot",
                                           name="rotq")
                            nc.vector.tensor_mul(rot[:], pr[:], ssinT[:, qsl])
                            nc.vector.tensor_add(qrT[:], qrT[:], rot[:])

                    # ---- C chunk: causal attention, software-pipelined ----
                    if h == 0:
                        transpose_batch(range(4 * i4, 4 * i4 + 4))
                    # units: (key tile j, qlo, qw, mask, first, last)
                    nj_off = 4 * i4
                    units = []
                    for j in range(nj_off):
                        units.append((j, 0, 512, None, j == 0, False))
                    units.append((nj_off + 0, 0, 256, dmasks[0],
                                  nj_off == 0, False))
                    units.append((nj_off + 1, 0, 256, dmasks[1], False, True))
                    units.append((nj_off + 0, 256, 256, None,
                                  nj_off == 0, False))
                    units.append((nj_off + 1, 256, 256, None, False, False))
                    units.append((nj_off + 2, 256, 256, dmasks[0],
                                  False, False))
                    units.append((nj_off + 3, 256, 256, dmasks[1],
                                  False, True))
                    psden = pden.tile([1, 512], F32, tag="den", name="psden")
                    psy = [ppy.tile([P, 512], F32, tag=f"y{dc}",
                                    name=f"psy{dc}")
                           for dc in range(KVC)]

                    def dav(pu):
                        (j, qlo, qw, _m, first, last), se = pu
                        qs = slice(qlo, qlo + qw)
                        nc.tensor.matmul(
                            psden[:, qs], ones_col[:], se[:, 0:qw],
                            start=first, stop=last, skip_group_check=True,
                        )
                        for dc in range(KVC):
                            nc.tensor.matmul(
                                psy[dc][:, qs],
                                v[:, j, dc * P : (dc + 1) * P], se[:, 0:qw],
                                start=first, stop=last, skip_group_check=True,
                            )

                    pend = None
                    for u in units:
                        (j, qlo, qw, mask, first, last) = u
                        kt, kr_i, ksl = kslice(j)
                        qs = slice(qlo, qlo + qw)
                        ps = pp.tile([P, 512], F32, tag="mm", name="psS")
                        for dc in range(KVC):
                            nc.tensor.matmul(
                                ps[:, 0:qw], kt[:, kr_i, dc, ksl],
                                qTc[dc][:, qs],
                                start=(dc == 0), stop=False,
                            )
                        nc.tensor.matmul(
                            ps[:, 0:qw], kt[:RH, kr_i, KVC, ksl], qrT[:, qs],
                            start=False, stop=True,
                        )
                        se = bcs.tile([P, 512], BF16, tag="se", bufs=3,
                                      name="se")
                        nc.scalar.activation(se[:, 0:qw], ps[:, 0:qw], AF.Exp)
                        if mask is not None:
                            nc.vector.tensor_mul(se[:, 0:qw], se[:, 0:qw],
                                                 mask[:])
                        if pend is not None:
                            dav(pend)
                        pend = (u, se)
                    dav(pend)

                    deninv = bcs.tile([1, 512], F32, tag="deninv",
                                      name="deninv")
                    nc.vector.reciprocal_approx_fast(out=deninv[:],
                                                     in_=psden[:])
                    denb = bcs.tile([P, 512], F32, tag="denb", name="denb")
                    nc.gpsimd.partition_broadcast(denb[:], deninv[:])
                    for dc in range(KVC):
                        nc.vector.tensor_mul(
                            yT[:, h * KVC + dc, qsl], psy[dc][:], denb[:]
                        )

        # ================= Phase D: out^T = W_out_c^T @ y^T ===============
        with ExitStack() as dctx:
            dwp = dctx.enter_context(tc.tile_pool(name="dwp", bufs=2))
            dst = dctx.enter_context(tc.tile_pool(name="dst", bufs=2))
            ppD = dctx.enter_context(tc.tile_pool(name="ppD", bufs=2,
                                                  space="PSUM"))

            DK = HPC * KV // P  # 8 contraction chunks
            for mc in range(E // P):
                wot = dwp.tile([P, DK, P], BF16, tag="wo", name="wo")
                nc.sync.dma_start(wot[:], wo[mc])
                psD = [ppD.tile([P, 512], F32, tag=f"d{tcc}", name=f"psD{tcc}")
                       for tcc in range(NT512)]
                for kc in range(DK):
                    for tcc in range(NT512):
                        nc.tensor.matmul(
                            psD[tcc][:], wot[:, kc, :],
                            yT[:, kc, tcc * 512 : (tcc + 1) * 512],
                            start=(kc == 0), stop=(kc == DK - 1),
                        )
                ost = dst.tile([P, T], BF16, tag="ost", name="ost")
                for tcc in range(NT512):
                    osl = slice(tcc * 512, (tcc + 1) * 512)
                    if tcc % 2 == 0:
                        nc.vector.tensor_copy(ost[:, osl], psD[tcc][:])
                    else:
                        nc.scalar.copy(ost[:, osl], psD[tcc][:])
                eng = nc.gpsimd if mc % 2 == 0 else nc.scalar
                eng.dma_start(outT[mc], ost[:])

    nc.compile()
    return nc


_NC_CACHE = {}


def _get_nc(T=T_FULL):
    if T not in _NC_CACHE:
        _NC_CACHE[T] = build_kernel(T)
    return _NC_CACHE[T]


def _swizzle_k(w, p=P):
    K, M = w.shape
    return np.ascontiguousarray(w.reshape(K // p, p, M).transpose(1, 0, 2))


def make_in_maps(x, cos, sin, W_qkv, W_qdec, W_out):
    import ml_dtypes

    bf16 = ml_dtypes.bfloat16
    xT = np.ascontiguousarray(np.asarray(x)[0].T).astype(bf16)
    cosT = np.ascontiguousarray(np.asarray(cos).T.astype(np.float32))
    sinT = np.ascontiguousarray(np.asarray(sin).T.astype(np.float32))
    sinN = sinT.copy()
    sinN[: RH // 2, :] *= -1.0
    W_qkv = np.asarray(W_qkv).astype(bf16)
    wcq = _swizzle_k(W_qkv[:, QKH:])
    wkv = _swizzle_k(W_qkv[:, :QKH])
    W_qdec = np.asarray(W_qdec)
    W_out = np.asarray(W_out)
    in_maps = []
    for c in range(NCORES):
        tsl = slice(c * TC, (c + 1) * TC)
        wos = W_out[c * HPC * KV : (c + 1) * HPC * KV].astype(bf16)
        wos = _swizzle_k(wos)                      # [128, 8, 2048]
        wos = wos.reshape(P, HPC * KV // P, E // P, P)
        wos = np.ascontiguousarray(wos.transpose(2, 0, 1, 3))  # [16,128,8,128]
        in_maps.append({
            "xT": _swizzle_k(np.ascontiguousarray(xT[:, tsl])),
            "wcq": wcq,
            "wkv": wkv,
            "wqd": _swizzle_k(np.ascontiguousarray(
                W_qdec[:, c * HPC * QKH : (c + 1) * HPC * QKH]).astype(bf16)),
            "wo": wos,
            "cosT": cosT.astype(bf16),
            "sinT": sinN.astype(bf16),
            "cosl": np.ascontiguousarray(cosT[:, tsl]),
            "sinl": np.ascontiguousarray(sinN[:, tsl]),
        })
    return in_maps


def kernel(x, cos, sin, W_qkv, W_qdec, W_out, _trace=False, _tmpdir=None):
    T = np.asarray(x).shape[1]
    nc = _get_nc(T)
    in_maps = make_in_maps(x, cos, sin, W_qkv, W_qdec, W_out)
    res = run_bass_kernel_spmd(
        nc, in_maps, core_ids=list(range(NCORES)),
        trace=_trace, tmpdir=_tmpdir,
    )
    out = np.zeros((E, T), np.float32)
    for r in res.results:
        out += np.asarray(r["outT"], dtype=np.float32).reshape(E, T)
    kernel.last_results = res
    return np.ascontiguousarray(out.T)[None].astype(np.float32)
